# revision 52
# baseline (speedup 1.0000x reference)
"""Binarized 3x3 conv (XNOR-style): sign(conv2d(sign(x), sign(w)) + b).

Full-input contract: kernel(x=[32,256,56,56]f32, weight=[256,256,3,3]f32,
bias=[256]f32) -> [32,256,56,56]f32.

Strategy: data-parallel over batch across 8 NeuronCores (4 images/core).
Per core:
  - sign(x) encoded as +/-0.5 (exact: is_ge -> {0,1}, subtract 0.5) into
    zero-padded 58-col rows, fp8e4. Each image is split into four row
    bands (padded rows 0-9 / 8-17 / 16-33 / 32-57, 2-row halos) so the
    first matmul block only needs one 9-row chunk per ci half and input
    chunks pipeline against the matmuls. Rows are stored at a 64-byte
    pitch so every row (and the DoubleRow ci stride) is 16B aligned.
  - all sign conversion on the vector engine (gpsimd is far too slow for
    bulk conversion; it only does the small border memsets). The xf32
    staging pool is kept shallow (bufs=3) so early input DMAs are not
    bandwidth-shared with a deep prefetch queue.
  - sign(w) prepped on host as +/-1 in [c_partition, kg, tap, pair, k]
    layout.
  - conv = 9 tap-shifted matmuls per 8-row block (fp8 DoubleRow,
    contract=256) accumulating into PSUM. The moving AP is 4D
    [p, ci=2, row=8, col=56], skipping the 2 pad columns of each row, so
    only the 448 useful columns stream (not 464). psum == conv/2 exactly.
  - output sign via the scalar engine: Sign(psum [+ bias/2]) -> +/-1/0,
    exact for the half-integer psum values, written as bf16 (exact for
    {-1,0,1}) to halve store traffic; the host upcasts to f32. Stores go
    out via the scalar engine's DMA queues (keeps sync/HWDGE loads
    unblocked).
  - a short PE warmup matmul chain opens the HAM clock gate while the
    first chunks stream in.
"""

import numpy as np

import concourse.bacc as bacc
import concourse.mybir as mybir
import concourse.tile as tile
from concourse.bass_utils import run_bass_kernel_spmd

N_CORES = 8
N_PER = 4          # images per core
C = 256            # input channels
K = 256            # output channels
H = W = 56
WP = 58            # padded row data width (56 + 2 halo cols)
RP = 64            # row pitch: every row 16B-aligned; cols 58-63 unused
RB = 8             # output rows per matmul tile
F = RB * W         # 448 matmul free size (8 rows x 56, no pad cols)
NBLK = H // RB     # 7 row blocks per image

N_WARM = 12

# bands: (base padded row, rows); ci stride rows*64 is always %16
BANDS = [(0, 10), (8, 10), (16, 18), (32, 26)]
# chunks: (band, orig_r0, n_rows, band_row0)
CHUNKS = [(0, 0, 9, 1), (1, 7, 10, 0), (2, 15, 18, 0), (3, 31, 25, 0)]
# row block -> (band, local row offset)
RB_MAP = [(0, 0), (1, 0), (2, 0), (2, 8), (3, 0), (3, 8), (3, 16)]

_cache = {}


def _build(with_bias):
    dt = mybir.dt
    xdt = dt.float8e4
    nc = bacc.Bacc()
    x_d = nc.declare_dram_parameter("xs", [N_PER, C, H, W], dt.float32, isOutput=False)
    wfree = 9 * 2 * 256
    w_d = nc.declare_dram_parameter("wsgn", [128, wfree], xdt, isOutput=False)
    if with_bias:
        b_d = nc.declare_dram_parameter("bhalf", [128, 2], dt.float32, isOutput=False)
    o_d = nc.declare_dram_parameter("out", [N_PER, K, H, W], dt.bfloat16, isOutput=True)

    with tile.TileContext(nc) as tc:
        with (
            tc.tile_pool(name="wpool", bufs=1) as wpool,
            tc.tile_pool(name="xsgn", bufs=N_PER) as xsgn_pool,
            tc.tile_pool(name="xf32", bufs=3) as xf_pool,
            tc.tile_pool(name="osb", bufs=6) as o_pool,
            tc.tile_pool(name="psum", bufs=8, space="PSUM") as p_pool,
        ):
            # Warm the PE HAM clock gate while the first chunks stream in.
            wsrc = wpool.tile([128, 512], xdt)
            nc.gpsimd.memset(wsrc[:], 0.0)
            warm = p_pool.tile([128, F], dt.float32, tag="ps")
            for _ in range(N_WARM):
                nc.tensor.matmul(
                    warm[:], wsrc[:, 0:128], wsrc[:, 0:F],
                    start=True, stop=True,
                )
            w_sb = wpool.tile([128, wfree], xdt)
            if with_bias:
                b_sb = wpool.tile([128, 2], dt.float32)
                nc.sync.dma_start(b_sb[:], b_d[:])

            bands = [[None] * len(BANDS) for _ in range(N_PER)]
            xv = x_d[:].rearrange("n c h w -> n c (h w)")

            def emit_borders(n, bi, ci):
                base, nr = BANDS[bi]
                xt = bands[n][bi]
                o = ci * nr * RP
                rowv = xt[:, o: o + nr * RP].rearrange("p (h w) -> p h w", w=RP)
                nc.gpsimd.memset(rowv[:, :, 0:1], 0.0)    # left halo col
                nc.gpsimd.memset(rowv[:, :, 57:58], 0.0)  # right halo col
                if bi == 0:
                    nc.gpsimd.memset(xt[:, o + 1: o + 57], 0.0)  # top border
                if bi == 3:
                    nc.gpsimd.memset(                            # bottom border
                        xt[:, o + (nr - 1) * RP + 1: o + (nr - 1) * RP + 57], 0.0)

            def emit_chunk(n, c, ci, fast=False):
                bi, r0, nr, br0 = CHUNKS[c]
                xt = bands[n][bi]
                xf = xf_pool.tile([128, nr * W], dt.float32, tag="xf32",
                                  name=f"xf_{n}_{bi}_{ci}")
                src = xv[n, ci * 128:(ci + 1) * 128, r0 * W: (r0 + nr) * W]
                # image 0's ci1 loads issue from the scalar engine so the
                # two descriptor chains run in parallel during the lead-in
                eng = nc.scalar if fast else nc.sync
                eng.dma_start(xf[:], src)
                dst = (
                    xt[:].rearrange("p (i r c) -> p i r c", i=2, c=RP)
                    [:, ci, br0: br0 + nr, 1:57]
                )
                src = xf[:].rearrange("p (h w) -> p h w", h=nr)
                # (x>=0 -> {0,1}) - 0.5 = +/-0.5, exact
                nc.vector.tensor_scalar(
                    dst, src, 0.0, 0.5, mybir.AluOpType.is_ge,
                    mybir.AluOpType.subtract,
                )

            def make_bands(n):
                for bi, (base, nr) in enumerate(BANDS):
                    bands[n][bi] = xsgn_pool.tile(
                        [128, 2 * nr * RP], xdt, tag=f"b{bi}", name=f"b{bi}_{n}")

            # image 0: first band's chunks first, second weight half after
            # the second band (needed by the second matmul group)
            make_bands(0)
            for bi in range(len(BANDS)):
                for ci in range(2):
                    emit_borders(0, bi, ci)
            # image 0: first bands' chunks first; the shallow xf pool keeps
            # later loads out of flight so the critical ones get the HBM
            # bandwidth share
            nc.sync.dma_start(w_sb[:, 0: wfree // 2], w_d[:, 0: wfree // 2])
            for c in (0, 1):
                emit_chunk(0, c, 0)
                emit_chunk(0, c, 1, fast=True)
            nc.scalar.dma_start(w_sb[:, wfree // 2:], w_d[:, wfree // 2:])
            for c in (2, 3):
                emit_chunk(0, c, 0)
                emit_chunk(0, c, 1, fast=True)
            for n in range(1, N_PER):
                make_bands(n)
                for bi in range(len(BANDS)):
                    for ci in range(2):
                        emit_borders(n, bi, ci)
                for c in range(len(CHUNKS)):
                    emit_chunk(n, c, 0)
                    emit_chunk(n, c, 1)

            wv = w_sb[:].rearrange("p (g t i k) -> p g t i k", g=2, t=9, i=2)

            def emit_rb(n, kg, rb, r0=0, nr=RB, last=False):
                fr = nr * W
                ps = p_pool.tile([128, fr], dt.float32, tag="ps",
                                 name=f"ps{kg}_{n}_{rb}_{r0}",
                                 padded_shape=[128, F])
                bi, l0 = RB_MAP[rb]
                vt = bands[n][bi][:].rearrange(
                    "p (i r c) -> p i r c", i=2, c=RP)
                for tap in range(9):
                    ty, tx = tap // 3, tap % 3
                    nc.tensor.matmul(
                        ps[:], wv[:, kg, tap, :, :],
                        vt[:, :, l0 + r0 + ty: l0 + r0 + ty + nr, tx: tx + W],
                        start=(tap == 0), stop=(tap == 8),
                        perf_mode=mybir.MatmulPerfMode.DoubleRow,
                    )
                # sign evacuation on the scalar engine: psum holds conv/2
                # (half-integers), Sign(v [+ b/2]) is the exact output
                osb = o_pool.tile([128, fr], dt.bfloat16, tag="osb",
                                  name=f"osb{kg}_{n}_{rb}_{r0}",
                                  padded_shape=[128, F])
                if with_bias:
                    nc.scalar.activation(
                        osb[:], ps[:], mybir.ActivationFunctionType.Sign,
                        bias=b_sb[:, kg: kg + 1],
                    )
                else:
                    nc.scalar.activation(
                        osb[:], ps[:], mybir.ActivationFunctionType.Sign)
                dst = o_d[n, kg * 128:(kg + 1) * 128,
                          rb * RB + r0: rb * RB + r0 + nr, :]
                if last:
                    # the final store sits on the tail critical path: halve
                    # it across two queues, with the descriptor chains on
                    # two different engines so they issue in parallel
                    nc.sync.dma_start(dst[0:64], osb[0:64])
                    nc.scalar.dma_start(dst[64:128], osb[64:128])
                else:
                    nc.scalar.dma_start(dst, osb[:])

            for n in range(N_PER):
                for rb in range(NBLK):
                    for kg in range(2):
                        if n == N_PER - 1 and rb == NBLK - 1 and kg == 1:
                            # split the last group so the final evacuation
                            # and store cover only 4 rows
                            emit_rb(n, kg, rb, 0, RB // 2)
                            emit_rb(n, kg, rb, RB // 2, RB // 2, last=True)
                        else:
                            emit_rb(n, kg, rb)

    nc.finalize()
    return nc


def _prep_weights(weight):
    sgn = np.sign(weight.astype(np.float32))
    w6 = sgn.reshape(2, 128, 2, 128, 3, 3)     # [kg, kk, i, p, ty, tx]
    arr = w6.transpose(3, 0, 4, 5, 2, 1)       # [p, kg, ty, tx, i, kk]
    arr = np.ascontiguousarray(arr).reshape(128, 9 * 2 * 256)
    return arr.astype(mybir.dt.np(mybir.dt.float8e4))


def kernel(x, weight, bias, _profile=False, _trace_kwargs=None):
    x = np.asarray(x, dtype=np.float32)
    weight = np.asarray(weight, dtype=np.float32)
    bias = np.asarray(bias, dtype=np.float32)
    assert x.shape == (N_CORES * N_PER, C, H, W), x.shape
    assert weight.shape == (K, C, 3, 3), weight.shape
    assert bias.shape == (K,), bias.shape
    with_bias = bool(np.any(bias != 0.0))

    if with_bias not in _cache:
        _cache[with_bias] = _build(with_bias)
    nc = _cache[with_bias]

    wsgn = _prep_weights(weight)
    in_maps = []
    for c in range(N_CORES):
        m = {
            "xs": np.ascontiguousarray(x[c * N_PER:(c + 1) * N_PER]),
            "wsgn": wsgn,
        }
        if with_bias:
            m["bhalf"] = np.ascontiguousarray(
                (bias.reshape(2, 128).T * 0.5).astype(np.float32)
            )
        in_maps.append(m)

    res = run_bass_kernel_spmd(
        nc, in_maps, core_ids=list(range(N_CORES)),
        trace=_profile, **(_trace_kwargs or {}),
    )
    out = np.concatenate(
        [np.asarray(res.results[c]["out"]).astype(np.float32)
         for c in range(N_CORES)], axis=0)
    if _profile:
        kernel.last_exec_ns = res.exec_time_ns
        kernel.last_results = res
    return out
